# revision 32
# baseline (speedup 1.0000x reference)
# Trainium2 Bass kernel: 2:4 structured activation pruning + Linear.
#
#   out = magnitude_prune_2of4(x.reshape(-1, 4096)) @ weight.T
#
# Sharding: data-parallel over the flattened token dim (16384 tokens ->
# 2048/core across 8 cores); weight replicated (host-transposed + bf16).
# No collectives.
#
# v4 pipeline (per 128-token tile):
#   DMA x (bf16, host-cast; host pre-zeroes bf16-tie-ambiguous groups so
#   device top-2 selection matches the reference exactly) -> DVE pairwise
#   min/max tree in bf16 with contiguous-pair addressing (2x DVE modes)
#   -> PRUNE24 select -> DMA XBAR transpose (SBUF->SBUF) -> PE matmul
#   bf16 accumulating 32 d-chunks into PSUM, c-outer loop so one
#   stationary load feeds both output halves -> ACT PSUM->SBUF -> DMA out.
import numpy as np

N_CORES = 8
BS, SEQ, D = 4, 4096, 4096
OUTF = 1024
TOK_TOTAL = BS * SEQ
TOK = TOK_TOTAL // N_CORES      # 2048 tokens per core
P = 128                         # SBUF partitions
NT = TOK // P                   # 16 token tiles per core
NCH = D // P                    # 32 d-chunks of 128
HALF = D // 2                   # 2048

_compiled = None
_custom_ops = None


def _register_custom_dve():
    # Fused DVE ops: pairwise abs-max/abs-min, and the pruning select
    # out = |x| >= thr ? x : 0.
    global _custom_ops
    if _custom_ops is not None:
        return _custom_ops
    from concourse import dve_ops as Dv
    from concourse.dve_spec import Spec, Src0, Src1, Zero, maxx, minn, select, lower
    from concourse.dve_uop import DveOpSpec

    def mk(name, body, reference):
        spec = Spec(body=body, reference=reference)
        shas = {}
        for ver in ("v3", "v4"):
            try:
                u = lower(spec, ver=ver)
                shas[ver] = DveOpSpec(name=name, opcode=1, uops=u,
                                      rd1_en=True).sha(ver)
            except Exception:
                if ver == "v3":
                    raise
        return Dv.DveOp(name=name, spec=spec, subdim=False, uops_sha=shas)

    absa = maxx(Src0, Zero - Src0)
    absb = maxx(Src1, Zero - Src1)
    ops = (
        mk("ABS_MAX2_ANT", maxx(absa, absb),
           lambda in0, in1: np.maximum(np.abs(in0), np.abs(in1))),
        mk("ABS_MIN2_ANT", minn(absa, absb),
           lambda in0, in1: np.minimum(np.abs(in0), np.abs(in1))),
        mk("PRUNE24_ANT", select(maxx(Src0, Zero - Src0) >= Src1, Src0, Zero),
           lambda in0, in1: np.where(np.abs(in0) >= in1, in0, 0.0)),
    )
    for op in ops:
        if op.name not in Dv._SUB_OPCODE_FOR_NAME:
            Dv.OPS.append(op)
            Dv.CUSTOM_DVE_SPECS[op.name] = op.spec
            Dv._SUB_OPCODE_FOR_NAME[op.name] = (
                Dv._CUSTOM_DVE_ROW_BASE + len(Dv._SUB_OPCODE_FOR_NAME))
    _custom_ops = ops
    return ops


def _build():
    import concourse.tile as tile
    import concourse.mybir as mybir
    from concourse import bacc

    ABS_MAX2, ABS_MIN2, PRUNE24 = _register_custom_dve()
    f32 = mybir.dt.float32
    bf16 = mybir.dt.bfloat16
    Alu = mybir.AluOpType

    nc = bacc.Bacc("TRN2", target_bir_lowering=False, debug=False,
                   num_devices=N_CORES)
    xs_ap = nc.dram_tensor("xs", [TOK, D], bf16, kind="ExternalInput").ap()
    wt_ap = nc.dram_tensor("wt", [D, OUTF], bf16, kind="ExternalInput").ap()
    o_ap = nc.dram_tensor("o", [TOK, OUTF], f32, kind="ExternalOutput").ap()

    with tile.TileContext(nc) as tc:
        with tc.tile_pool(name="wpool", bufs=1) as wpool, \
             tc.tile_pool(name="xin", bufs=4) as xin, \
             tc.tile_pool(name="mwork", bufs=1) as mwork, \
             tc.tile_pool(name="xsp", bufs=4) as xspp, \
             tc.tile_pool(name="xtp", bufs=4) as xtp, \
             tc.tile_pool(name="outp", bufs=4) as outp, \
             tc.tile_pool(name="pso", bufs=4, space="PSUM") as pso:

            # Warm the ACT function table immediately: a dummy 1-element
            # scalar copy forces the ACT_TABLE_LOAD fetch to sequence
            # before the bulk weight DMAs, so the real PSUM->SBUF copies
            # on the scalar queue never stall on the table mid-run.
            warm0 = wpool.tile([P, 1], f32, tag="warm0")
            warm1 = wpool.tile([P, 1], f32, tag="warm1")
            nc.vector.memset(warm0, 0.0)
            nc.scalar.copy(warm1, warm0)

            # weight.T resident in SBUF as bf16: [d-in-chunk, chunk, outf].
            # 4 large DMAs on the sync HWDGE queue (hardware descriptor
            # generation; gpsimd SWDGE was slow to materialize), emitted
            # interleaved with the early x loads and ordered to match the
            # early tiles' n-separated matmul consumption.
            w_sb = wpool.tile([P, NCH, OUTF], bf16)
            wt3 = wt_ap.rearrange("(c p) o -> p c o", p=P)

            def wload(n, ch):
                c0 = ch * (NCH // 2)
                nc.sync.dma_start(
                    out=w_sb[:, c0:c0 + NCH // 2, n * 512:(n + 1) * 512],
                    in_=wt3[:, c0:c0 + NCH // 2, n * 512:(n + 1) * 512])

            def load(i, spans):
                # x-in DMAs only, on the sync queue: nothing with upstream
                # compute dependencies ever sits in front of an x load.
                xh = xin.tile([P, D], bf16, tag="xh")
                for lo, w in spans:
                    nc.sync.dma_start(out=xh[:, lo:lo + w],
                                      in_=xs_ap[i * P:(i + 1) * P,
                                                lo:lo + w])
                return xh

            def math(xh, spans):
                # 2:4 threshold (bf16, host-consistent tie fix), prune,
                # XBAR transpose (scalar queue).
                xsp = xspp.tile([P, D], bf16, tag="xsp")
                xspT = xtp.tile([P, NCH, P], bf16, tag="xspT")
                for lo, w in spans:
                    ng = w // 4
                    xv = xh[:, lo:lo + w]
                    # iterate [pair-slot, group]: strided reads (free) but
                    # CONTIGUOUS packed writes into split halves, so the
                    # level-2/3 stock tensor_tensor ops see contiguous bf16
                    # operands and run in the packed 2x mode
                    xq = xv.rearrange("p (g h two) -> p h two g",
                                      h=2, two=2)
                    mx = mwork.tile([P, HALF], bf16, tag="mx")
                    mn = mwork.tile([P, HALF], bf16, tag="mn")
                    mxs = mx[:, :w // 2]
                    mns = mn[:, :w // 2]
                    mx_t = mxs.rearrange("p (two g) -> p two g", two=2)
                    mn_t = mns.rearrange("p (two g) -> p two g", two=2)
                    nc.vector._custom_dve(ABS_MAX2, out=mx_t,
                                          in0=xq[:, 0, :, :],
                                          in1=xq[:, 1, :, :])
                    nc.vector._custom_dve(ABS_MIN2, out=mn_t,
                                          in0=xq[:, 0, :, :],
                                          in1=xq[:, 1, :, :])
                    mm = mxs[:, :ng]
                    nm = mns[:, :ng]
                    nc.vector.tensor_tensor(mm, mxs[:, :ng], mxs[:, ng:],
                                            Alu.min)
                    nc.vector.tensor_tensor(nm, mns[:, :ng], mns[:, ng:],
                                            Alu.max)
                    thr = mm
                    nc.vector.tensor_tensor(thr, mm, nm, Alu.max)
                    thr_b = thr.unsqueeze(2).broadcast_to([P, ng, 4])
                    sp = xsp[:, lo:lo + w]
                    nc.vector._custom_dve(
                        PRUNE24,
                        out=sp.rearrange("p (g four) -> p g four", four=4),
                        in0=xv.rearrange("p (g four) -> p g four", four=4),
                        in1=thr_b)
                # XBAR transposes at half-tile granularity, on the sync
                # queue (scalar stays out of the early critical path)
                for lo in range(0, D, HALF):
                    nc.sync.dma_start(
                        out=xspT[:, lo // P:(lo + HALF) // P, :],
                        in_=xsp[:, lo:lo + HALF], transpose=True)
                return xspT

            def back(i, xspT, n_sep=False):
                # matmuls on PE: c-outer so each stationary xspT chunk is
                # loaded once and streamed against both outf halves.
                # n_sep (early tiles): finish outf-half 0 first so the
                # half-1 weight DMAs have more time to land.
                # PSUM->SBUF copy + out DMA on scalar queue.
                pout0 = pso.tile([P, OUTF // 2], f32, tag="pout0")
                pout1 = pso.tile([P, OUTF // 2], f32, tag="pout1")
                pouts = [pout0, pout1]

                def drain(n):
                    # PSUM->SBUF on scalar (table pre-warmed), DMA out.
                    # Keeping these off the Vector queue lets the DVE run
                    # ahead instead of stalling at a matmul-end barrier.
                    osb = outp.tile([P, OUTF // 2], f32)
                    nc.scalar.copy(osb, pouts[n])
                    nc.scalar.dma_start(
                        out=o_ap[i * P:(i + 1) * P, n * 512:(n + 1) * 512],
                        in_=osb)

                if n_sep:
                    # finish outf-half 0 first and drain it while half-1
                    # matmuls still run (shorter weight wait early /
                    # shorter tail on the last tile)
                    for n in range(2):
                        for c in range(NCH):
                            nc.tensor.matmul(
                                pouts[n], xspT[:, c, :],
                                w_sb[:, c, n * 512:(n + 1) * 512],
                                start=(c == 0), stop=(c == NCH - 1))
                        drain(n)
                else:
                    for c in range(NCH):
                        for n in range(2):
                            nc.tensor.matmul(
                                pouts[n], xspT[:, c, :],
                                w_sb[:, c, n * 512:(n + 1) * 512],
                                start=(c == 0), stop=(c == NCH - 1))
                    for n in range(2):
                        drain(n)

            # software pipeline: emit tile i+1's front before tile i's
            # matmul stage so every engine's FIFO order matches readiness
            # order (no head-of-line blocking behind matmul-dependent ops).
            quarters = [(q * (D // 4), D // 4) for q in range(4)]
            halves = [(0, HALF), (HALF, HALF)]

            def spans_of(i):
                if i == 0:
                    return quarters
                return halves if i < 3 else [(0, D)]

            # software pipeline, x loads running 2 tiles ahead:
            #   load(i+2) | math(i+1) | back(i)
            # weight quarters interleaved between the early x loads so the
            # sync queue issues them in consumption order.
            xhs = {0: load(0, spans_of(0))}   # tile-0 x ahead of weights
            wload(0, 0)
            xhs[1] = load(1, spans_of(1))
            wload(0, 1)
            wload(1, 0)
            prev = math(xhs.pop(0), spans_of(0))
            for i in range(1, NT):
                if i + 1 < NT:
                    xhs[i + 1] = load(i + 1, spans_of(i + 1))
                if i == 1:
                    wload(1, 1)
                cur = math(xhs.pop(i), spans_of(i))
                back(i - 1, prev, n_sep=(i - 1 < 2))
                prev = cur
            back(NT - 1, prev, n_sep=True)
    nc.compile()
    return nc


def _get_compiled():
    global _compiled
    if _compiled is None:
        _compiled = _build()
    return _compiled


def _fix_ties_bf16(x_flat):
    # Device selection: keep x_i iff bf16|x_i| >= (2nd-largest bf16|x| of
    # the group). bf16 rounding is monotone, so for groups whose 2nd and
    # 3rd bf16 magnitudes differ the kept SET equals the reference's
    # (top-2 by exact |x|, stable). For ambiguous groups (bf16 2nd == 3rd)
    # pre-zero the two reference-dropped elements: the device then sees
    # them as 0 and keeps exactly the reference pair. Zeroed elements are
    # dropped by the reference anyway, so values are unaffected.
    import ml_dtypes
    xb = x_flat.astype(ml_dtypes.bfloat16)
    b = np.abs(xb.astype(np.float32)).reshape(-1, 4)
    m1 = np.maximum(b[:, 0], b[:, 1]); n1 = np.minimum(b[:, 0], b[:, 1])
    m2 = np.maximum(b[:, 2], b[:, 3]); n2 = np.minimum(b[:, 2], b[:, 3])
    lo_hi = np.minimum(m1, m2); hi_lo = np.maximum(n1, n2)
    second = np.maximum(lo_hi, hi_lo)
    third = np.minimum(lo_hi, hi_lo)
    amb = np.flatnonzero(second == third)
    if len(amb):
        ge = np.abs(x_flat.reshape(-1, 4)[amb])
        order = np.argsort(-ge, axis=1, kind="stable")       # exact, stable
        gb = xb.reshape(-1, 4)
        rows = gb[amb]
        np.put_along_axis(rows, order[:, 2:], 0, axis=1)
        gb[amb] = rows
    return xb


def _prepare_in_maps(x, weight):
    import ml_dtypes
    x_flat = np.ascontiguousarray(x.reshape(TOK_TOTAL, D), dtype=np.float32)
    xb = _fix_ties_bf16(x_flat)
    wt = np.ascontiguousarray(weight.T, dtype=np.float32) \
        .astype(ml_dtypes.bfloat16)
    return [{"xs": xb[c * TOK:(c + 1) * TOK], "wt": wt}
            for c in range(N_CORES)]


def kernel(x: np.ndarray, weight: np.ndarray) -> np.ndarray:
    from concourse.bass_utils import run_bass_kernel_spmd

    nc = _get_compiled()
    in_maps = _prepare_in_maps(x, weight)
    res = run_bass_kernel_spmd(nc, in_maps, core_ids=list(range(N_CORES)))
    out = np.concatenate([res.results[c]["o"] for c in range(N_CORES)], axis=0)
    return out.reshape(BS, SEQ, OUTF)
